# revision 9
# baseline (speedup 1.0000x reference)
"""Trainium2 Bass kernel for the dual-branch cross-attention module.

Computation (see the module's reference):
    q1,k1,v1 = split(x @ w_qkv1); q2,k2,v2 = split(y @ w_qkv2)   (B,H,L,D)
    a1 = softmax(1 - q1 k2^T / sqrt(D));  xo = a1 @ v1
    a2 = softmax(1 - q2 k1^T / sqrt(D));  yo = a2 @ v2
    out = (xo @ w_p1 + b_p1, yo @ w_p2 + b_p2)

Sharding: batch*heads across 8 cores. Core c handles batch b=c//2 and the
8-head slice h0=(c%2)*8. Each core computes its full LxL attention and a
per-head-pair partial output projection over its 512 channels; the host sums
the pair partials and the two cores' partials per batch and adds the bias
(softmax(1-z) == softmax(-z), so the constant shift is dropped).

Device-side design notes:
  - Inputs are pre-transposed and cast to bf16 on the host (xT/yT layouts
    [128, 8, 1024]); no PE transposes and half the DMA traffic.
  - QKV runs tensor-major (q1, k2, v1 first) so branch-0 attention can start
    while k1/q2/v2 matmuls act as PE filler inside the ACT-paced attention
    windows.
  - Attention windows run entirely in the PE's (64,128) row-tiled mode: the
    two heads' S^T matmuls (K=64) execute concurrently on row tiles T0/T8,
    and each PV matmul is split into two K=64 halves (also T0/T8) whose
    partial accumulators are summed on DVE during normalization. No PE
    mode switches inside a window.
  - exp runs on ACT only ([128,1024] chunks); rowsums ride in the PV
    matmuls as a ones-column (M=65).
  - Output projection is emitted per (pair, branch) as soon as its onorm is
    ready, giving late PE filler; partials are written bf16 and summed on
    the host.

Self-contained: shapes/sharding hardcoded; imports only the system bass stack.
"""

import os
import sys
from contextlib import ExitStack

import numpy as np
import ml_dtypes

for _p in ("/opt/trn_rl_repo", os.path.expanduser("~/.axon_site/_ro/trn_rl_repo")):
    if os.path.isdir(_p) and _p not in sys.path:
        sys.path.insert(0, _p)

import concourse.tile as tile
from concourse import bacc, mybir
from concourse.bass_utils import run_bass_kernel_spmd

F32 = mybir.dt.float32
BF16 = mybir.dt.bfloat16
EXP = mybir.ActivationFunctionType.Exp
BF16_NP = ml_dtypes.bfloat16

L = 1024          # sequence length
DIM = 1024        # model dim
D = 64            # head dim
SCALE = D ** -0.5
PROJ = 256        # projection out dim
NCORES = 8
PAIRS = 4         # head pairs per core (8 heads / 2)
KC = 8            # contraction chunks of 128 over DIM
MC = 8            # key-position chunks of 128 over L
LWIN = 512        # window (psum-bank-limited matmul free dim)
NLW = L // LWIN

W_NAMES = ("wq1", "wk1", "wv1", "wq2", "wk2", "wv2")


def _build_body(nc, tc, ins, outs, ctx):
    big = ctx.enter_context(tc.tile_pool(name="big", bufs=1))
    qkp = ctx.enter_context(tc.tile_pool(name="qkp", bufs=1))
    ep = ctx.enter_context(tc.tile_pool(name="ep", bufs=5))
    onp = ctx.enter_context(tc.tile_pool(name="onp", bufs=1))
    smp = ctx.enter_context(tc.tile_pool(name="smp", bufs=3))
    outp = ctx.enter_context(tc.tile_pool(name="outp", bufs=2))
    st_ps = ctx.enter_context(tc.tile_pool(name="st_ps", bufs=2, space="PSUM"))
    pv_ps = ctx.enter_context(tc.tile_pool(name="pv_ps", bufs=2, space="PSUM"))

    # ---- persistent SBUF tiles, DMA'd directly in final layout ----
    xT = big.tile([128, KC, L], BF16, tag="xT")
    yT = big.tile([128, KC, L], BF16, tag="yT")
    w_t = {nm: big.tile([128, KC, 512], BF16, tag=nm, name=nm)
           for nm in W_NAMES}
    wp_t = {nm: big.tile([128, PAIRS, PROJ], BF16, tag=nm, name=nm)
            for nm in ("wp1", "wp2")}

    def load_chunks(names):
        # one DMA per 128-partition chunk, in consumption order
        for nm in names:
            dst = {"xT": xT, "yT": yT}.get(nm) or w_t.get(nm) or wp_t.get(nm)
            nch = dst.shape[1]
            for c in range(nch):
                nc.sync.dma_start(out=dst[:, c, :], in_=ins[nm][:, c, :])

    load_chunks(["wq1", "xT", "wk2", "yT", "wv1"])
    load_chunks(["wk1", "wq2", "wv2", "wp1", "wp2"])

    qk = {}     # (nm, pair) -> [128, L] bf16 (rows 0:64 head A, 64:128 head B)
    vaug = {}   # (pair, branch) -> [128, MC, 130] bf16 (V + ones cols)
    onorm = {}  # (pair, branch) -> [128, L] bf16 normalized O^T

    def emit_qk_group(nm, p):
        """One (tensor, pair): 16 matmuls + 1 evac; st-pool slot."""
        src = xT if nm in ("q1", "k1") else yT
        wt = w_t["w" + nm]
        cols = slice(p * 128, (p + 1) * 128)
        dstT = qkp.tile([128, L], BF16, tag=f"{nm}_{p}", name=f"qk_{nm}_{p}")
        qk[(nm, p)] = dstT
        mm = st_ps.tile([128, L], F32, tag="st", name="mm_qk")
        for lw in range(NLW):
            for c in range(KC):
                nc.tensor.matmul(
                    mm[:, lw * LWIN:(lw + 1) * LWIN],
                    wt[:, c, cols], src[:, c, lw * LWIN:(lw + 1) * LWIN],
                    start=(c == 0), stop=(c == KC - 1),
                )
        nc.vector.tensor_copy(out=dstT, in_=mm)

    def emit_v_group(br, lt):
        """One l-tile of the V projection: 8 matmuls + 4 strided evacs."""
        nm, src = ("wv1", xT) if br == 0 else ("wv2", yT)
        wt = w_t[nm]
        if lt == 0:
            for p in range(PAIRS):
                va = onp.tile([128, MC, 130], BF16, tag=f"va_{p}_{br}",
                              name=f"va_{p}_{br}")
                nc.vector.memset(va[:, :, 64:65], 1.0)
                nc.vector.memset(va[:, :, 129:130], 1.0)
                vaug[(p, br)] = va
        mm = st_ps.tile([128, L], F32, tag="st", name="mm_v")
        for c in range(KC):
            nc.tensor.matmul(
                mm[:, 0:512], src[:, c, lt * 128:(lt + 1) * 128], wt[:, c, :],
                start=(c == 0), stop=(c == KC - 1),
            )
        for p in range(PAIRS):
            va = vaug[(p, br)]
            # [128, 2, 64] strided copy: head A -> cols 0:64, head B -> 65:129
            nc.vector.tensor_copy(
                out=va[:, lt, :].rearrange("p (h n) -> p h n", h=2)[:, :, 0:64],
                in_=mm[:, p * 128:(p + 1) * 128].rearrange("p (h n) -> p h n", h=2),
            )

    def emit_proj_group(p, br):
        """Projection partial for one (pair, branch): 2 slots x 4 matmuls."""
        wp_nm, out_nm = (("wp1", "p1"), ("wp2", "p2"))[br]
        wt = wp_t[wp_nm]
        on = onorm[(p, br)]
        for half in range(4):
            # one matmul group per PSUM bank (bank-aligned offsets 0 and 512)
            mm = st_ps.tile([128, L], F32, tag="st", name="mm_pr")
            for i in range(2):
                lt = half * 2 + i
                nc.tensor.matmul(
                    mm[:, i * 512:i * 512 + PROJ],
                    on[:, lt * 128:(lt + 1) * 128], wt[:, p, :],
                    start=True, stop=True,
                )
            ob = outp.tile([128, 2, PROJ], BF16, tag="ob", name="ob")
            nc.vector.tensor_copy(
                out=ob,
                in_=mm.rearrange("p (i n) -> p i n", i=2)[:, :, 0:PROJ])
            nc.sync.dma_start(
                out=outs[out_nm][p][:, half * 2:(half + 1) * 2, :], in_=ob)

    # ---- attention ----
    def window(p, br, lw, filler):
        """One 512-wide query window of unit (pair, branch); all matmuls in
        (64,128) row-tiled mode. `filler` is a generator of emit-callbacks."""
        qT = qk[("q1", p)] if br == 0 else qk[("q2", p)]
        kT = qk[("k2", p)] if br == 0 else qk[("k1", p)]
        va = vaug[(p, br)]
        on = onorm[(p, br)]
        lsl = slice(lw * LWIN, (lw + 1) * LWIN)
        pvA = pv_ps.tile([65, 512], F32, tag="pvA", name="pvA")
        pvB = pv_ps.tile([65, 512], F32, tag="pvB", name="pvB")
        es = {}

        def emit_s(mc):
            msl = slice(mc * 128, (mc + 1) * 128)
            st = st_ps.tile([128, 1024], F32, tag="st", name="st")
            nc.tensor.matmul(st[:, 0:512], kT[0:64, msl], qT[0:64, lsl],
                             start=True, stop=True)
            nc.tensor.matmul(st[:, 512:1024], kT[64:128, msl], qT[64:128, lsl],
                             start=True, stop=True)
            e_t = ep.tile([128, 1024], BF16, tag="E", name="E")
            es[mc] = e_t
            nc.scalar.activation(out=e_t, in_=st, func=EXP, scale=-SCALE)

        def emit_pv(mc):
            e_t = es.pop(mc)
            st_, sp_ = (mc == 0), (mc == MC - 1)
            nc.tensor.matmul(pvA, va[:, mc, 0:65], e_t[:, 0:512],
                             start=st_, stop=sp_)
            nc.tensor.matmul(pvB, va[:, mc, 65:130], e_t[:, 512:1024],
                             start=st_, stop=sp_)

        emit_s(0)
        emit_s(1)
        for mc in range(MC):
            emit_pv(mc)
            if mc + 2 < MC:
                emit_s(mc + 2)
            if mc in (2, 5):
                for cb in filler:
                    cb()
                    break

        # normalize: rb = bcast(1/rowsum) then onorm = pv[0:64] * rb
        for head, pv in ((0, pvA), (1, pvB)):
            ssum = smp.tile([1, 512], F32, tag="ssum", name="ssum")
            nc.vector.tensor_copy(out=ssum, in_=pv[64:65, :])
            rr = smp.tile([1, 512], F32, tag="rr", name="rr")
            nc.vector.reciprocal_approx_fast(out=rr, in_=ssum)
            rb = smp.tile([64, 512], F32, tag="rb", name="rb")
            nc.gpsimd.partition_broadcast(rb, rr)
            nc.vector.tensor_mul(out=on[head * 64:(head + 1) * 64, lsl],
                                 in0=pv[0:64, :], in1=rb)

    # ---- emission schedule ----
    def filler_gen():
        for nm in ("k1", "q2"):
            for p in range(PAIRS):
                yield (lambda nm=nm, p=p: emit_qk_group(nm, p))
        for lt in range(MC):
            yield (lambda lt=lt: emit_v_group(1, lt))
        # proj groups become available as units complete; emitted directly
        # in the unit loop below.

    for nm in ("q1", "k2"):
        for p in range(PAIRS):
            emit_qk_group(nm, p)
    for lt in range(MC):
        emit_v_group(0, lt)

    fill = filler_gen()
    units = [(p, 0) for p in range(PAIRS)] + [(p, 1) for p in range(PAIRS)]
    for ui, (p, br) in enumerate(units):
        on = onp.tile([128, L], BF16, tag=f"on_{p}_{br}", name=f"on_{p}_{br}")
        onorm[(p, br)] = on
        for lw in range(NLW):
            window(p, br, lw, fill)
        emit_proj_group(p, br)
    # drain any unused filler (shouldn't happen, but be safe)
    for cb in fill:
        cb()


def build():
    nc = bacc.Bacc("TRN2", target_bir_lowering=False, debug=False,
                   num_devices=NCORES)
    ins = {}
    for nm in ("xT", "yT"):
        ins[nm] = nc.dram_tensor(nm, [128, KC, L], BF16,
                                 kind="ExternalInput").ap()
    for nm in W_NAMES:
        ins[nm] = nc.dram_tensor(nm, [128, KC, 512], BF16,
                                 kind="ExternalInput").ap()
    for nm in ("wp1", "wp2"):
        ins[nm] = nc.dram_tensor(nm, [128, PAIRS, PROJ], BF16,
                                 kind="ExternalInput").ap()
    outs = {}
    for nm in ("p1", "p2"):
        # per-pair partials [pair][l (as p i), proj]
        t = nc.dram_tensor(nm, [PAIRS, L, PROJ], BF16, kind="ExternalOutput").ap()
        outs[nm] = [t[pp].rearrange("(i p) n -> p i n", p=128)
                    for pp in range(PAIRS)]
    with tile.TileContext(nc) as tc:
        with ExitStack() as ctx:
            _build_body(nc, tc, ins, outs, ctx)
    nc.compile()
    return nc


_NC_CACHE = None


def _get_nc():
    global _NC_CACHE
    if _NC_CACHE is None:
        _NC_CACHE = build()
    return _NC_CACHE


def _chunk128(w):
    """[1024, N] -> [128, 8, N] with (p, c, n) = w[c*128+p, n]."""
    n = w.shape[1]
    return np.ascontiguousarray(
        w.reshape(KC, 128, n).transpose(1, 0, 2)).astype(BF16_NP)


def make_in_maps(x, y, w_qkv1, w_qkv2, w_p1, w_p2):
    """Shard + pre-transpose the full inputs: core c -> batch c//2,
    head-slice (c%2)*8."""
    xTs = []
    yTs = []
    for b in range(4):
        xTs.append(_chunk128(np.ascontiguousarray(x[b].T).reshape(DIM, L)))
        yTs.append(_chunk128(np.ascontiguousarray(y[b].T).reshape(DIM, L)))
    halves = []
    for half in range(2):
        c0 = half * 512
        m = {}
        for wsrc, names in ((w_qkv1, ("wq1", "wk1", "wv1")),
                            (w_qkv2, ("wq2", "wk2", "wv2"))):
            for j, nm in enumerate(names):
                base = j * DIM + c0
                m[nm] = _chunk128(np.ascontiguousarray(wsrc[:, base:base + 512]))
        for wp, nm in ((w_p1, "wp1"), (w_p2, "wp2")):
            m[nm] = np.ascontiguousarray(
                wp[c0:c0 + 512, :].reshape(PAIRS, 128, PROJ)
                .transpose(1, 0, 2)).astype(BF16_NP)
        halves.append(m)
    in_maps = []
    for c in range(NCORES):
        b, half = divmod(c, 2)
        m = dict(halves[half])
        m["xT"] = xTs[b]
        m["yT"] = yTs[b]
        in_maps.append(m)
    return in_maps


def run_cores(in_maps, trace=False, trace_cores=None):
    nc = _get_nc()
    return run_bass_kernel_spmd(nc, in_maps, list(range(NCORES)),
                                trace=trace, trace_cores=trace_cores)


def kernel(x, y, w_qkv1, w_qkv2, w_p1, b_p1, w_p2, b_p2):
    x = np.asarray(x, dtype=np.float32)
    y = np.asarray(y, dtype=np.float32)
    in_maps = make_in_maps(x, y, np.asarray(w_qkv1), np.asarray(w_qkv2),
                           np.asarray(w_p1), np.asarray(w_p2))
    res = run_cores(in_maps).results
    def tot(c, nm):
        return np.asarray(res[c][nm], dtype=np.float32).sum(axis=0)
    out1 = np.stack([tot(2 * b, "p1") + tot(2 * b + 1, "p1") for b in range(4)])
    out2 = np.stack([tot(2 * b, "p2") + tot(2 * b + 1, "p2") for b in range(4)])
    out1 += np.asarray(b_p1, dtype=np.float32)
    out2 += np.asarray(b_p2, dtype=np.float32)
    return out1, out2
